# revision 1
# baseline (speedup 1.0000x reference)
"""Trainium2 Bass kernel for nn_BESNumEigen3qubitModel.

Math reduction (exact):
  vec = rho_vec / ||rho_vec||;  rho = sum_i vec_i G_i + I/8  (Hermitian 8x8, trace 1)
  dm0 = beta0*(rho - I/8) + I/8, dm1 = beta1*(rho - I/8) + I/8 are AFFINE in rho,
  and partial transposes are linear, so every eigvalsh in the reference reduces
  to eigenvalues of just 3 Hermitian matrices per batch element:
     rho, pt_a(rho), pt_c(rho).
  With w = eig(rho) ascending, S_k0 = sum of k0 smallest, T_k1 = sum of k1 largest,
  mu/nu = eig extrema of pt_a/pt_c:
     beta0 = 1/(1-8 w_min), beta1 = 1/(1-8 w_max)   (beta0>0, beta1<0)
     loss0 = beta0*(S_k0 - k0/8) + k0/8 ; loss1 = beta1*(T_k1 - k1/8) + k1/8
     loss  = (loss0+loss1)^2 + sum over 4 PPT terms (beta*(ext-1/8)+1/8)^2
  where ext = mu_min (beta0), mu_max (beta1), nu_min (beta0), nu_max (beta1).

Device kernel: batched branchless complex Jacobi (4 full sweeps, XOR-pair order)
on 3*4096 = 12288 8x8 Hermitian matrices per core (batch on partitions, matrices
along free dim), then an 8-element sorting network on rho's diagonal, min/max
reduction for the PT diagonals, and the scalar loss assembly.
"""

import numpy as np

D = 8
BATCH = 32768
NCORES = 8
PER_CORE = BATCH // NCORES       # 4096
NTILES = PER_CORE // 128         # 32 batch tiles per core
NM = 3 * NTILES                  # 96 matrices per partition (type-major)

_f32 = np.float32

# ---------------------------------------------------------------- host prep --

def _gellmann_basis(d):
    mats = []
    for j in range(d):
        for k in range(j + 1, d):
            m = np.zeros((d, d), np.complex128); m[j, k] = 1; m[k, j] = 1
            mats.append(m)
    for j in range(d):
        for k in range(j + 1, d):
            m = np.zeros((d, d), np.complex128); m[j, k] = -1j; m[k, j] = 1j
            mats.append(m)
    for l in range(1, d):
        m = np.zeros((d, d), np.complex128)
        m[np.arange(l), np.arange(l)] = 1
        m[l, l] = -l
        mats.append(np.sqrt(2.0 / (l * (l + 1))) * m)
    return np.stack(mats)


def _build_maps():
    """[64, 384] f32 map: (vec, 1) -> 128 floats each of rho, pt_a(rho), pt_c(rho).

    Float layout per matrix: f in [0,64) = Re[i,j] at f=i*8+j; [64,128) = Im[i,j].
    """
    G = _gellmann_basis(D)
    B = np.zeros((64, 128), np.float64)
    for k in range(63):
        B[k, :64] = G[k].real.reshape(-1)
        B[k, 64:] = G[k].imag.reshape(-1)
    B[63, :64] = (np.eye(D) / D).reshape(-1)

    def entry_perm(kind):
        p = np.zeros(64, np.int64)
        for i in range(8):
            for j in range(8):
                if kind == 'a':
                    i2, j2 = (j & 4) | (i & 3), (i & 4) | (j & 3)
                else:
                    i2, j2 = (i & 6) | (j & 1), (j & 6) | (i & 1)
                p[i * 8 + j] = i2 * 8 + j2
        return p

    def float_perm(kind):
        e = entry_perm(kind)
        return np.concatenate([e, 64 + e])

    M3 = np.concatenate([B, B[:, float_perm('a')], B[:, float_perm('c')]], axis=1)
    return M3.astype(_f32)


_M3 = None


def _host_prep(rho_vec):
    global _M3
    if _M3 is None:
        _M3 = _build_maps()
    vec = rho_vec.astype(np.float64)
    vec = vec / np.linalg.norm(vec, axis=-1, keepdims=True)
    vec_aug = np.concatenate(
        [vec.astype(_f32), np.ones((vec.shape[0], 1), _f32)], axis=1)
    flat = vec_aug @ _M3                                   # [B, 384]
    arr = flat.reshape(NCORES, NTILES, 128, 3, 128)        # [core, t, p, type, f]
    return [np.ascontiguousarray(
        arr[c].transpose(1, 2, 0, 3).reshape(128, NM * 128)) for c in range(NCORES)]


# ------------------------------------------------------------ device kernel --

def _xor_pairs(r):
    return [(i, i ^ r) for i in range(8) if i < (i ^ r)]


# Batcher odd-even mergesort network for 8 elements (19 comparators)
_CE8 = [(0, 1), (2, 3), (4, 5), (6, 7), (0, 2), (1, 3), (4, 6), (5, 7),
        (1, 2), (5, 6), (0, 4), (1, 5), (2, 6), (3, 7), (2, 4), (3, 5),
        (1, 2), (3, 4), (5, 6)]

N_SWEEPS = 4


def _build_program(k0, k1):
    import concourse.bass as bass
    import concourse.bacc as bacc
    import concourse.mybir as mybir
    from concourse.tile import TileContext
    from contextlib import ExitStack

    f32 = mybir.dt.float32
    ALU = mybir.AluOpType
    ACT = mybir.ActivationFunctionType

    nc = bacc.Bacc("TRN2")
    mats_d = nc.dram_tensor("mats", [128, NM * 128], f32, kind="ExternalInput")
    out_d = nc.dram_tensor("out", [128, NTILES], f32, kind="ExternalOutput")

    with ExitStack() as ctx:
        tc = ctx.enter_context(TileContext(nc))
        main = ctx.enter_context(tc.tile_pool(name="main", bufs=1))
        pp = ctx.enter_context(tc.tile_pool(name="pp", bufs=3))
        cp = ctx.enter_context(tc.tile_pool(name="cp", bufs=3))

        A = main.tile([128, NM, 128], f32, name="A")
        for ch in range(8):
            nc.sync.dma_start(
                out=A[:, ch * 12:(ch + 1) * 12, :],
                in_=mats_d[:, ch * 12 * 128:(ch + 1) * 12 * 128])

        A4 = A[:].rearrange("p m (i j) -> p m i j", i=16, j=8)
        eps30 = main.tile([128, 1], f32, name="eps30")
        nc.vector.memset(eps30[:], 1e-30)
        eps35 = main.tile([128, 1], f32, name="eps35")
        nc.vector.memset(eps35[:], 1e-35)
        SH = [128, NM, 8]

        def P(tag):
            return pp.tile([128, NM], f32, tag=tag, name=tag)[:]

        def C(tag):
            return cp.tile(SH, f32, tag=tag, name=tag)[:]

        def emit_rotation(p, q, M):
            app = A4[:, 0:M, p, p]
            aqq = A4[:, 0:M, q, q]
            X = A4[:, 0:M, p, q]
            Y = A4[:, 0:M, 8 + p, q]
            SH16 = [128, M, 16]

            def PM(tag):
                return pp.tile([128, NM], f32, tag=tag, name=tag)[:][:, 0:M]

            def C16(tag):
                return cp.tile([128, NM, 16], f32, tag=tag, name=tag)

            Aap = A[:]
            pdim = list(Aap.ap[0])

            def swap_col(col):
                # [im-half; re-half] view of column `col`: [128, M, 2, 8]
                return bass.AP(tensor=Aap.tensor, offset=Aap.offset + 64 + col,
                               ap=[pdim, [128, M], [-64, 2], [8, 8]])

            sqx, sqy, n2p, g = PM("sqx"), PM("sqy"), PM("n2p"), PM("g")
            gsq, s2, h, ag = PM("gsq"), PM("s2"), PM("h"), PM("ag")
            den, T, sg, T2 = PM("den"), PM("T"), PM("sg"), PM("T2")
            t2, cden, c, u = PM("t2"), PM("cden"), PM("c"), PM("u")
            urb2, sr, si, v1 = PM("urb2"), PM("sr"), PM("si"), PM("v1")
            tb, dpp, dqq, nsr = PM("tb"), PM("dpp"), PM("dqq"), PM("nsr")
            csi_t = pp.tile([128, NM, 2], f32, tag="csi", name="csi")
            csi = csi_t[:][:, 0:M, :]

            nc.scalar.activation(sqx, X, ACT.Square, scale=2.0)
            nc.scalar.activation(sqy, Y, ACT.Square, scale=2.0)
            nc.vector.tensor_tensor(n2p, sqx, sqy, ALU.add)        # b'^2 = 4|apq|^2
            nc.vector.tensor_tensor(g, app, aqq, ALU.subtract)     # g' = app - aqq
            nc.scalar.square(gsq, g)
            nc.vector.tensor_tensor(s2, gsq, n2p, ALU.add)
            nc.scalar.activation(h, s2, ACT.Sqrt, bias=eps30[:])   # sqrt(g^2+b'^2)
            nc.scalar.activation(ag, g, ACT.Abs)
            nc.vector.tensor_tensor(den, ag, h, ALU.add)
            nc.vector.reciprocal(T, den)                           # 1/(|g|+h)
            nc.scalar.sign(sg, g, bias=eps35[:])                   # sign(g), 0 -> +1
            nc.gpsimd.tensor_tensor(T2, T, T, ALU.mult)
            nc.gpsimd.tensor_tensor(t2, n2p, T2, ALU.mult)         # t^2
            nc.scalar.activation(cden, t2, ACT.Sqrt, bias=1.0)     # sqrt(1+t^2)
            nc.vector.reciprocal(c, cden)                          # cos
            nc.gpsimd.tensor_tensor(u, T, sg, ALU.mult)
            nc.vector.scalar_tensor_tensor(urb2, u, 2.0, c, ALU.mult, ALU.mult)
            nc.gpsimd.tensor_tensor(sr, urb2, X, ALU.mult)
            nc.gpsimd.tensor_tensor(si, urb2, Y, ALU.mult)
            nc.vector.tensor_tensor(v1, T, n2p, ALU.mult)
            nc.vector.scalar_tensor_tensor(tb, v1, 0.5, sg, ALU.mult, ALU.mult)
            nc.gpsimd.tensor_tensor(dpp, app, tb, ALU.add)
            nc.gpsimd.tensor_tensor(dqq, aqq, tb, ALU.subtract)
            nc.scalar.activation(nsr, sr, ACT.Copy, scale=-1.0)
            nc.gpsimd.tensor_copy(csi[:, :, 0], si)
            nc.scalar.activation(csi[:, :, 1], si, ACT.Copy, scale=-1.0)

            Ap16 = A4[:, 0:M, 0:16, p]
            Aq16 = A4[:, 0:M, 0:16, q]
            Aqsw = swap_col(q)
            cp16_t, P1_t, P2_t = C16("cp16"), C16("P1"), C16("P2")
            Q1_t, Q2_t = C16("Q1"), C16("Q2")
            cp16 = cp16_t[:][:, 0:M, :]
            P1 = P1_t[:][:, 0:M, :]
            P2 = P2_t[:][:, 0:M, :]
            Q1 = Q1_t[:][:, 0:M, :]
            Q2 = Q2_t[:][:, 0:M, :]
            P2h = P2.rearrange("p m (h j) -> p m h j", h=2)
            Q2h = Q2.rearrange("p m (h j) -> p m h j", h=2)
            cpap = cp16_t[:]
            cpsw = bass.AP(tensor=cpap.tensor, offset=cpap.offset + 8,
                           ap=[list(cpap.ap[0]), [16, M], [-8, 2], [1, 8]])

            cb16 = c[:, :, None].to_broadcast(SH16)
            srb16 = sr[:, :, None].to_broadcast(SH16)
            nsrb16 = nsr[:, :, None].to_broadcast(SH16)
            csb = csi[:, :, :, None].to_broadcast([128, M, 2, 8])
            TT = nc.vector.tensor_tensor
            GT = nc.gpsimd.tensor_tensor

            nc.scalar.copy(cp16, Ap16)               # old col p (re;im)
            GT(P1, srb16, Aq16, ALU.mult)            # [sr*Aqre ; sr*Aqim]
            TT(P2h, csb, Aqsw, ALU.mult)             # [si*Aqim ; -si*Aqre]
            TT(Ap16, cb16, Ap16, ALU.mult)
            TT(Ap16, Ap16, P1, ALU.add)
            TT(Ap16, Ap16, P2, ALU.add)
            GT(Q1, nsrb16, cp16, ALU.mult)           # [-sr*cpre ; -sr*cpim]
            GT(Q2h, csb, cpsw, ALU.mult)             # [si*cpim ; -si*cpre]
            TT(Aq16, cb16, Aq16, ALU.mult)
            TT(Aq16, Aq16, Q1, ALU.add)
            TT(Aq16, Aq16, Q2, ALU.add)
            # Hermitian row restore: row = conj(new col)
            nc.scalar.copy(A4[:, 0:M, p, 0:8], A4[:, 0:M, 0:8, p])
            nc.scalar.activation(A4[:, 0:M, 8 + p, 0:8], A4[:, 0:M, 8:16, p], ACT.Copy, scale=-1.0)
            nc.scalar.copy(A4[:, 0:M, q, 0:8], A4[:, 0:M, 0:8, q])
            nc.scalar.activation(A4[:, 0:M, 8 + q, 0:8], A4[:, 0:M, 8:16, q], ACT.Copy, scale=-1.0)
            # diagonal + annihilated entries
            nc.gpsimd.tensor_copy(A4[:, 0:M, p, p], dpp)
            nc.gpsimd.tensor_copy(A4[:, 0:M, q, q], dqq)
            nc.gpsimd.memset(A4[:, 0:M, 8 + p, p], 0.0)
            nc.gpsimd.memset(A4[:, 0:M, 8 + q, q], 0.0)
            nc.scalar.memzero(A4[:, 0:M, p, q])
            nc.scalar.memzero(A4[:, 0:M, 8 + p, q])
            nc.scalar.memzero(A4[:, 0:M, q, p])
            nc.scalar.memzero(A4[:, 0:M, 8 + q, p])

        for s in range(N_SWEEPS):
            M = NM if s < N_SWEEPS - 1 else NTILES   # last sweep: rho only
            for r in range(1, 8):
                for (p, q) in _xor_pairs(r):
                    emit_rotation(p, q, M)

        # ---- rho diagonal sort (matrices m in [0, NTILES)) ----
        tmin = main.tile([128, NTILES], f32, name="tmin")[:]
        for (i, j) in _CE8:
            di = A4[:, 0:NTILES, i, i]
            dj = A4[:, 0:NTILES, j, j]
            nc.vector.tensor_tensor(tmin, di, dj, ALU.min)
            nc.vector.tensor_tensor(dj, di, dj, ALU.max)
            nc.gpsimd.tensor_copy(di, tmin)

        # ---- pt_a / pt_c diagonal min/max (m in [NTILES, 3*NTILES)) ----
        dv = main.tile([128, 2 * NTILES, 8], f32, name="dv")
        for k in range(8):
            nc.gpsimd.tensor_copy(dv[:, :, k], A4[:, NTILES:NM, k, k])
        mn = main.tile([128, 2 * NTILES], f32, name="mn")[:]
        mx = main.tile([128, 2 * NTILES], f32, name="mx")[:]
        nc.vector.tensor_reduce(mn, dv[:], mybir.AxisListType.X, ALU.min)
        nc.vector.tensor_reduce(mx, dv[:], mybir.AxisListType.X, ALU.max)
        mu_min = mn[:, 0:NTILES]
        mu_max = mx[:, 0:NTILES]
        nu_min = mn[:, NTILES:2 * NTILES]
        nu_max = mx[:, NTILES:2 * NTILES]

        # ---- loss assembly ----
        def L(name):
            return main.tile([128, NTILES], f32, tag=name, name=name)[:]

        w_min = A4[:, 0:NTILES, 0, 0]
        w_max = A4[:, 0:NTILES, 7, 7]
        b0, b1, acc, t1, t2_, t3 = L("b0"), L("b1"), L("acc"), L("t1"), L("t2"), L("t3")

        nc.vector.tensor_scalar(b0, w_min, -8.0, 1.0, ALU.mult, ALU.add)
        nc.vector.reciprocal(b0, b0)
        nc.vector.tensor_scalar(b1, w_max, -8.0, 1.0, ALU.mult, ALU.add)
        nc.vector.reciprocal(b1, b1)

        # S_k0 = sum of k0 smallest, T_k1 = sum of k1 largest
        assert 1 <= k0 <= 8 and 1 <= k1 <= 8
        nc.gpsimd.tensor_copy(t1, A4[:, 0:NTILES, 0, 0])
        for i in range(1, k0):
            nc.vector.tensor_tensor(t1, t1, A4[:, 0:NTILES, i, i], ALU.add)
        nc.gpsimd.tensor_copy(t2_, A4[:, 0:NTILES, 7, 7])
        for i in range(6, 7 - k1, -1):
            nc.vector.tensor_tensor(t2_, t2_, A4[:, 0:NTILES, i, i], ALU.add)
        # loss0 = b0*(S_k0 - k0/8) + k0/8 ; loss1 = b1*(T_k1 - k1/8) + k1/8
        nc.vector.tensor_scalar(t1, t1, -k0 / 8.0, None, ALU.add)
        nc.vector.tensor_tensor(t1, t1, b0, ALU.mult)
        nc.vector.tensor_scalar(t2_, t2_, -k1 / 8.0, None, ALU.add)
        nc.vector.tensor_tensor(t2_, t2_, b1, ALU.mult)
        nc.vector.tensor_tensor(t1, t1, t2_, ALU.add)
        nc.vector.tensor_scalar(t1, t1, (k0 + k1) / 8.0, None, ALU.add)  # l01
        nc.vector.tensor_tensor(acc, t1, t1, ALU.mult)
        for beta, ext in ((b0, mu_min), (b1, mu_max), (b0, nu_min), (b1, nu_max)):
            nc.vector.tensor_scalar(t3, ext, -0.125, None, ALU.add)
            nc.vector.tensor_tensor(t3, t3, beta, ALU.mult)
            nc.vector.tensor_scalar(t3, t3, 0.125, None, ALU.add)
            nc.vector.tensor_tensor(t3, t3, t3, ALU.mult)
            nc.vector.tensor_tensor(acc, acc, t3, ALU.add)

        nc.sync.dma_start(out=out_d[:, :], in_=acc)

    nc.finalize()
    return nc


_prog_cache = {}


def kernel(rho_vec, rank0, rank1):
    rho_vec = np.asarray(rho_vec, dtype=np.float32)
    k0 = D - int(rank0)
    k1 = D - int(rank1)
    in_arrs = _host_prep(rho_vec)

    from concourse.bass_utils import run_bass_kernel_spmd
    key = (k0, k1)
    if key not in _prog_cache:
        _prog_cache[key] = _build_program(k0, k1)
    nc = _prog_cache[key]
    res = run_bass_kernel_spmd(
        nc, [{"mats": a} for a in in_arrs], core_ids=list(range(NCORES)))
    return np.concatenate(
        [np.asarray(res.results[c]["out"]).T.reshape(-1) for c in range(NCORES)]
    ).astype(np.float32)



# revision 5
# speedup vs baseline: 1.7899x; 1.7899x over previous
"""Trainium2 Bass kernel for nn_BESNumEigen3qubitModel (v2).

Same math reduction as v1 (eigenvalues of rho, pt_a(rho), pt_c(rho) per batch
element drive the whole loss), with a faster device algorithm:

  - column-major float layout (f = 64*h + 8*j + i for re/im h of entry (i,j))
    so matrix columns are unit-stride runs -> DVE 2x fp16 mode applies
  - matrices stored fp16, diagonals in a separate f32 tensor
  - 2 full Jacobi sweeps (all 3*32 matrices) + 1 rho-only sweep + a one-shot
    second-order diagonal correction for rho (replaces a 4th sweep)
  - per-round batched rotation-parameter chain (all 4 XOR pairs at once via
    affine access patterns), per-pair column rotations + Hermitian row restore
"""

import numpy as np

D = 8
BATCH = 32768
NCORES = 8
PER_CORE = BATCH // NCORES       # 4096
NTILES = PER_CORE // 128         # 32 rho matrices per partition
NM = 3 * NTILES                  # 96 matrices per partition (type-major)

_f32 = np.float32


# ---------------------------------------------------------------- host prep --

def _gellmann_basis(d):
    mats = []
    for j in range(d):
        for k in range(j + 1, d):
            m = np.zeros((d, d), np.complex128); m[j, k] = 1; m[k, j] = 1
            mats.append(m)
    for j in range(d):
        for k in range(j + 1, d):
            m = np.zeros((d, d), np.complex128); m[j, k] = -1j; m[k, j] = 1j
            mats.append(m)
    for l in range(1, d):
        m = np.zeros((d, d), np.complex128)
        m[np.arange(l), np.arange(l)] = 1
        m[l, l] = -l
        mats.append(np.sqrt(2.0 / (l * (l + 1))) * m)
    return np.stack(mats)


def _pt(m, kind):
    if kind == 'a':
        return np.swapaxes(m.reshape(2, 4, 2, 4), 1, 3).reshape(8, 8)
    return np.swapaxes(m.reshape(4, 2, 4, 2), 1, 3).reshape(8, 8)


def _build_maps():
    """[64, 3*128] f32 map (vec,1) -> col-major floats of rho/pt_a/pt_c,
    and [64, 3*8] diagonal map."""
    G = _gellmann_basis(D)
    basis = list(G) + [np.eye(D) / D]
    M3 = np.zeros((64, 3 * 128), np.float64)
    MD = np.zeros((64, 3 * 8), np.float64)
    for k, A in enumerate(basis):
        for t, At in enumerate((A, _pt(A, 'a'), _pt(A, 'c'))):
            # col-major: f = 64*h + 8*j + i
            M3[k, t * 128:t * 128 + 64] = At.real.T.reshape(-1)
            M3[k, t * 128 + 64:t * 128 + 128] = At.imag.T.reshape(-1)
            MD[k, t * 8:(t + 1) * 8] = np.diagonal(At).real
    return M3.astype(_f32), MD.astype(_f32)


_MAPS = None


def _host_prep(rho_vec):
    global _MAPS
    if _MAPS is None:
        _MAPS = _build_maps()
    M3, MD = _MAPS
    vec = rho_vec.astype(np.float64)
    vec = vec / np.linalg.norm(vec, axis=-1, keepdims=True)
    vec_aug = np.concatenate(
        [vec.astype(_f32), np.ones((vec.shape[0], 1), _f32)], axis=1)
    flat = (vec_aug @ M3).astype(np.float16)               # [B, 384]
    dflat = vec_aug @ MD                                   # [B, 24] f32
    arr = flat.reshape(NCORES, NTILES, 128, 3, 128)
    darr = dflat.reshape(NCORES, NTILES, 128, 3, 8)
    ins = []
    for c in range(NCORES):
        m = np.ascontiguousarray(
            arr[c].transpose(1, 2, 0, 3).reshape(128, NM * 128))
        dg = np.ascontiguousarray(
            darr[c].transpose(1, 2, 0, 3).reshape(128, NM * 8).astype(_f32))
        ins.append({"mats": m, "diag": dg})
    return ins


# ------------------------------------------------------------ device kernel --

def _msb(r):
    return 4 if r >= 4 else (2 if r >= 2 else 1)


# Batcher odd-even mergesort network for 8 elements (19 comparators)
_CE8 = [(0, 1), (2, 3), (4, 5), (6, 7), (0, 2), (1, 3), (4, 6), (5, 7),
        (1, 2), (5, 6), (0, 4), (1, 5), (2, 6), (3, 7), (2, 4), (3, 5),
        (1, 2), (3, 4), (5, 6)]


def _build_program(k0, k1):
    import concourse.bass as bass
    import concourse.bacc as bacc
    import concourse.mybir as mybir
    from concourse.tile import TileContext
    from contextlib import ExitStack

    f32 = mybir.dt.float32
    f16 = mybir.dt.float16
    ALU = mybir.AluOpType
    ACT = mybir.ActivationFunctionType

    nc = bacc.Bacc("TRN2")
    mats_d = nc.dram_tensor("mats", [128, NM * 128], f16, kind="ExternalInput")
    diag_d = nc.dram_tensor("diag", [128, NM * 8], f32, kind="ExternalInput")
    out_d = nc.dram_tensor("out", [128, NTILES], f32, kind="ExternalOutput")

    with ExitStack() as ctx:
        tc = ctx.enter_context(TileContext(nc))
        main = ctx.enter_context(tc.tile_pool(name="main", bufs=1))
        pp = ctx.enter_context(tc.tile_pool(name="pp", bufs=2))
        ep = ctx.enter_context(tc.tile_pool(name="ep", bufs=2))
        cp = ctx.enter_context(tc.tile_pool(name="cp", bufs=2))

        A = main.tile([128, NM, 128], f16, name="A")
        Dg = main.tile([128, NM, 8], f32, name="Dg")
        for ch in range(8):
            nc.sync.dma_start(
                out=A[:, ch * 12:(ch + 1) * 12, :],
                in_=mats_d[:, ch * 12 * 128:(ch + 1) * 12 * 128])
        nc.sync.dma_start(out=Dg[:, :, :], in_=diag_d[:, :])

        eps30 = main.tile([128, 1], f32, name="eps30")
        nc.vector.memset(eps30[:], 1e-30)
        eps35 = main.tile([128, 1], f32, name="eps35")
        nc.vector.memset(eps35[:], 1e-35)

        Aap = A[:]
        pdim = list(Aap.ap[0])
        Dap = Dg[:]
        dpdim = list(Dap.ap[0])

        def aAP(off, dims, M):
            return bass.AP(tensor=Aap.tensor, offset=Aap.offset + off,
                           ap=[list(pdim), [128, M], *[list(d) for d in dims]])

        def dAP(off, dims, M):
            return bass.AP(tensor=Dap.tensor, offset=Dap.offset + off,
                           ap=[list(dpdim), [8, M], *[list(d) for d in dims]])

        def tAP(t, off, dims, M):
            # custom inner-dim view of a [128, NM, ...] tile's [0:M] slice
            tap = t[:]
            return bass.AP(tensor=tap.tensor, offset=tap.offset + off,
                           ap=[list(tap.ap[0]), [tap.ap[1][0], M],
                               *[list(d) for d in dims]])

        TT = nc.vector.tensor_tensor
        GT = nc.gpsimd.tensor_tensor
        STT = nc.vector.scalar_tensor_tensor
        GSTT = nc.gpsimd.scalar_tensor_tensor

        # ---------------- one Jacobi round -------------------------------
        def emit_params(r, M, Dv_off, A_off, Mmats, tbdst=None, pre=""):
            """Rotation params for round r over M matrices.

            Returns (csm_t, srp_t, sip_t) tiles [128, Mmats, 4] f32 holding
            per-pair c / sr / si. If tbdst is None, Dg is updated in place
            (dpp/dqq); else tb is accumulated into tbdst AP pair positions.
            """
            hi = _msb(r)
            b1, b2 = [b for b in (1, 2, 4) if b != hi][::-1]  # b1 outer (larger)
            sg_ = lambda b: -b if (r & b) else b
            app = dAP(Dv_off, [[b1, 2], [b2, 2]], M)
            aqq = dAP(Dv_off + r, [[sg_(b1), 2], [sg_(b2), 2]], M)
            sX = [b + 8 * sg_(b) for b in (b1, b2)]
            X = aAP(A_off + 8 * r, [[sX[0], 2], [sX[1], 2]], M)
            Y = aAP(A_off + 8 * r + 64, [[sX[0], 2], [sX[1], 2]], M)

            tiles = {}

            def pt_(tag):
                t = pp.tile([128, Mmats, 4], f32, tag=pre + tag, name=pre + tag)
                tiles[tag] = t
                return (tAP(t, 0, [[2, 2], [1, 2]], M),   # [M, 2, 2] view
                        tAP(t, 0, [[1, 4]], M))           # flat [M, 4] view

            xx, xxf = pt_("xx")
            yy, yyf = pt_("yy")
            m2, m2f = pt_("m2")
            g, gf = pt_("g")
            g2, g2f = pt_("g2")
            s2, s2f = pt_("s2")
            rs, rsf = pt_("rs")
            h, hf = pt_("h")
            ag, agf = pt_("ag")
            den, denf = pt_("den")
            T, Tf = pt_("T")
            sgn, sgnf = pt_("sgn")
            v, vf = pt_("v")
            tb, tbf = pt_("tb")

            TT(xx, X, X, ALU.mult)
            nc.scalar.activation(yy, Y, ACT.Square)
            TT(m2f, xxf, yyf, ALU.add)
            GT(g, app, aqq, ALU.subtract)
            nc.scalar.activation(g2f, gf, ACT.Square)
            STT(s2f, m2f, 4.0, g2f, ALU.mult, ALU.add)
            nc.scalar.activation(rsf, s2f, ACT.Abs_reciprocal_sqrt,
                                 bias=eps30[:])
            nc.scalar.activation(agf, gf, ACT.Abs)
            TT(hf, s2f, rsf, ALU.mult)
            TT(denf, agf, hf, ALU.add)
            nc.vector.reciprocal(Tf, denf)
            nc.scalar.sign(sgnf, gf, bias=eps35[:])
            GT(vf, m2f, Tf, ALU.mult)
            GT(vf, vf, vf, ALU.add)              # v = 2*m2*T
            GT(tbf, vf, sgnf, ALU.mult)          # tb = 2*m2*T*sg (full)

            if tbdst is None:
                GT(app, app, tb, ALU.add)        # Dg[p] += tb
                GT(aqq, aqq, tb, ALU.subtract)   # Dg[q] -= tb
                T2, T2f = pt_("T2")
                t2, t2f = pt_("t2")
                csm, csmf = pt_("csm")
                u2, u2f = pt_("u2")
                urb2, urb2f = pt_("urb2")
                srp, srpf = pt_("srp")
                sip, sipf = pt_("sip")
                nc.scalar.activation(T2f, Tf, ACT.Square)
                STT(t2f, T2f, 4.0, m2f, ALU.mult, ALU.mult)
                nc.scalar.activation(csmf, t2f, ACT.Abs_reciprocal_sqrt,
                                     bias=1.0)
                GT(u2f, Tf, sgnf, ALU.mult)
                STT(urb2f, u2f, 2.0, csmf, ALU.mult, ALU.mult)
                TT(srp, urb2, X, ALU.mult)
                TT(sip, urb2, Y, ALU.mult)
                return tiles["csm"], tiles["srp"], tiles["sip"]
            else:
                tbp = tbdst(0, [[b1, 2], [b2, 2]])
                tbq = tbdst(r, [[sg_(b1), 2], [sg_(b2), 2]])
                GT(tbp, tbp, tb, ALU.add)
                GT(tbq, tbq, tb, ALU.subtract)
                return None

        def emit_round(r, M, Mc):
            """Round r: params over M matrices, col updates over Mc."""
            hi = _msb(r)
            b1, b2 = [b for b in (1, 2, 4) if b != hi][::-1]
            sg_ = lambda b: -b if (r & b) else b
            pairs = [(a, a ^ r) for a in (0, b2, b1, b1 + b2)]

            csm_t, srp_t, sip_t = emit_params(r, M, 0, 0, NM)

            # expansions over the 8-run (packed last dim for DVE 2x)
            c8t = ep.tile([128, NM, 4, 8], f16, tag="c8", name="c8")
            sr8t = ep.tile([128, NM, 4, 8], f16, tag="sr8", name="sr8")
            si8t = ep.tile([128, NM, 4, 8], f16, tag="si8", name="si8")

            def bc8(t, Mx):
                return tAP(t, 0, [[1, 4], [0, 8]], Mx)

            nc.scalar.copy(c8t[:][:, 0:Mc], bc8(csm_t, Mc))
            nc.vector.tensor_copy(sr8t[:][:, 0:Mc], bc8(srp_t, Mc))
            nc.vector.tensor_copy(si8t[:][:, 0:Mc], bc8(sip_t, Mc))

            def scal8(t, k, Mx):
                # [Mx, 2, 8] broadcast of per-pair scalar plane over halves
                return tAP(t, 8 * k, [[0, 2], [1, 8]], Mx)

            # ---------------- per-pair column phase ----------------------
            for k, (p, q) in enumerate(pairs):
                d_ = q - p
                colv = lambda c: aAP(8 * c, [[64, 2], [1, 8]], Mc)
                colJ = lambda c: aAP(8 * c + 64, [[-64, 2], [1, 8]], Mc)
                PCt = cp.tile([128, NM, 2, 16], f16, tag="PC", name="PC")
                T1t = cp.tile([128, NM, 2, 16], f16, tag="T1", name="T1")
                T2t = cp.tile([128, NM, 2, 16], f16, tag="T2", name="T2")

                slot = lambda t, s: tAP(t, 16 * s, [[8, 2], [1, 8]], Mc)
                TT(slot(T1t, 0), scal8(sr8t, k, Mc), colv(q), ALU.mult)
                TT(slot(T1t, 1), scal8(sr8t, k, Mc), colv(p), ALU.mult)
                TT(slot(T2t, 0), scal8(si8t, k, Mc), colJ(q), ALU.mult)
                TT(slot(T2t, 1), scal8(si8t, k, Mc), colJ(p), ALU.mult)
                TT(slot(PCt, 0), scal8(c8t, k, Mc), colv(p), ALU.mult)
                TT(slot(PCt, 1), scal8(c8t, k, Mc), colv(q), ALU.mult)
                # W+ on sel {(0,re),(1,im)}; W- on sel {(0,im),(1,re)}
                TT(tAP(T1t, 0, [[24, 2], [1, 8]], Mc),
                   tAP(T1t, 0, [[24, 2], [1, 8]], Mc),
                   tAP(T2t, 0, [[24, 2], [1, 8]], Mc), ALU.add)
                TT(tAP(T1t, 8, [[8, 2], [1, 8]], Mc),
                   tAP(T1t, 8, [[8, 2], [1, 8]], Mc),
                   tAP(T2t, 8, [[8, 2], [1, 8]], Mc), ALU.subtract)
                # finals into A columns
                TT(colv(p), slot(PCt, 0), slot(T1t, 0), ALU.add)
                TT(colv(q), slot(PCt, 1), slot(T1t, 1), ALU.subtract)
                # Hermitian row restore
                nc.scalar.copy(aAP(p, [[d_, 2], [8, 8]], Mc),
                               aAP(8 * p, [[8 * d_, 2], [1, 8]], Mc))
                nc.scalar.activation(aAP(64 + p, [[d_, 2], [8, 8]], Mc),
                                     aAP(64 + 8 * p, [[8 * d_, 2], [1, 8]], Mc),
                                     ACT.Copy, scale=-1.0)

            # ---------------- round-end fixes ----------------------------
            sX = [b + 8 * sg_(b) for b in (b1, b2)]
            ddp = dAP(0, [[b1, 2], [b2, 2]], Mc)
            ddq = dAP(r, [[sg_(b1), 2], [sg_(b2), 2]], Mc)
            nc.gpsimd.tensor_copy(aAP(0, [[9 * b1, 2], [9 * b2, 2]], Mc), ddp)
            nc.gpsimd.tensor_copy(
                aAP(9 * r, [[9 * sg_(b1), 2], [9 * sg_(b2), 2]], Mc), ddq)
            nc.gpsimd.memset(aAP(64, [[9 * b1, 2], [9 * b2, 2]], Mc), 0.0)
            nc.gpsimd.memset(
                aAP(64 + 9 * r, [[9 * sg_(b1), 2], [9 * sg_(b2), 2]], Mc), 0.0)
            # annihilate (p,q) and (q,p), re+im
            sQ = [8 * b + sg_(b) for b in (b1, b2)]
            nc.gpsimd.memset(aAP(8 * r, [[sX[0], 2], [sX[1], 2]], Mc), 0.0)
            nc.gpsimd.memset(aAP(8 * r + 64, [[sX[0], 2], [sX[1], 2]], Mc), 0.0)
            nc.gpsimd.memset(aAP(r, [[sQ[0], 2], [sQ[1], 2]], Mc), 0.0)
            nc.gpsimd.memset(aAP(r + 64, [[sQ[0], 2], [sQ[1], 2]], Mc), 0.0)

        # ---------------- sweeps ----------------------------------------
        for r in range(1, 8):
            emit_round(r, NM, NM)
        for r in range(1, 8):
            emit_round(r, NM, NM if r < 7 else NTILES)
        for r in range(1, 8):
            emit_round(r, NTILES, NTILES)

        # ---------------- one-shot rho correction ------------------------
        TB = main.tile([128, NTILES, 8], f32, name="TB")
        nc.vector.memset(TB[:], 0.0)
        Tap = TB[:]
        tpdim = list(Tap.ap[0])

        for r in range(1, 8):
            def tbdst(off, dims):
                return bass.AP(tensor=Tap.tensor, offset=Tap.offset + off,
                               ap=[list(tpdim), [8, NTILES],
                                   *[list(d) for d in dims]])
            emit_params(r, NTILES, 0, 0, NTILES, tbdst=tbdst, pre="k")

        TT(Dg[:, 0:NTILES, :], Dg[:, 0:NTILES, :], TB[:], ALU.add)

        # ---------------- rho diagonal sort ------------------------------
        tmin = main.tile([128, NTILES], f32, name="tmin")[:]
        dg8 = Dg[:]
        for (i, j) in _CE8:
            di = dg8[:, 0:NTILES, i]
            dj = dg8[:, 0:NTILES, j]
            TT(tmin, di, dj, ALU.min)
            TT(dj, di, dj, ALU.max)
            nc.gpsimd.tensor_copy(di, tmin)

        # ---------------- pt_a / pt_c min & max --------------------------
        mn = main.tile([128, 2 * NTILES], f32, name="mn")[:]
        mx = main.tile([128, 2 * NTILES], f32, name="mx")[:]
        ptd = dg8[:, NTILES:NM, :]
        nc.vector.tensor_reduce(mn, ptd, mybir.AxisListType.X, ALU.min)
        nc.vector.tensor_reduce(mx, ptd, mybir.AxisListType.X, ALU.max)
        mu_min = mn[:, 0:NTILES]
        mu_max = mx[:, 0:NTILES]
        nu_min = mn[:, NTILES:2 * NTILES]
        nu_max = mx[:, NTILES:2 * NTILES]

        # ---------------- loss assembly ----------------------------------
        def L(name):
            return main.tile([128, NTILES], f32, tag=name, name=name)[:]

        w_min = dg8[:, 0:NTILES, 0]
        w_max = dg8[:, 0:NTILES, 7]
        b0, b1_, acc, t1, t2_, t3 = (L("b0"), L("b1"), L("acc"), L("t1"),
                                     L("t2x"), L("t3"))

        nc.vector.tensor_scalar(b0, w_min, -8.0, 1.0, ALU.mult, ALU.add)
        nc.vector.reciprocal(b0, b0)
        nc.vector.tensor_scalar(b1_, w_max, -8.0, 1.0, ALU.mult, ALU.add)
        nc.vector.reciprocal(b1_, b1_)

        assert 1 <= k0 <= 8 and 1 <= k1 <= 8
        nc.gpsimd.tensor_copy(t1, dg8[:, 0:NTILES, 0])
        for i in range(1, k0):
            TT(t1, t1, dg8[:, 0:NTILES, i], ALU.add)
        nc.gpsimd.tensor_copy(t2_, dg8[:, 0:NTILES, 7])
        for i in range(6, 7 - k1, -1):
            TT(t2_, t2_, dg8[:, 0:NTILES, i], ALU.add)
        nc.vector.tensor_scalar(t1, t1, -k0 / 8.0, None, ALU.add)
        TT(t1, t1, b0, ALU.mult)
        nc.vector.tensor_scalar(t2_, t2_, -k1 / 8.0, None, ALU.add)
        TT(t2_, t2_, b1_, ALU.mult)
        TT(t1, t1, t2_, ALU.add)
        nc.vector.tensor_scalar(t1, t1, (k0 + k1) / 8.0, None, ALU.add)
        TT(acc, t1, t1, ALU.mult)
        for beta, ext in ((b0, mu_min), (b1_, mu_max), (b0, nu_min),
                          (b1_, nu_max)):
            nc.vector.tensor_scalar(t3, ext, -0.125, None, ALU.add)
            TT(t3, t3, beta, ALU.mult)
            nc.vector.tensor_scalar(t3, t3, 0.125, None, ALU.add)
            TT(t3, t3, t3, ALU.mult)
            TT(acc, acc, t3, ALU.add)

        nc.sync.dma_start(out=out_d[:, :], in_=acc)

    nc.finalize()
    return nc


_prog_cache = {}


def kernel(rho_vec, rank0, rank1):
    rho_vec = np.asarray(rho_vec, dtype=np.float32)
    k0 = D - int(rank0)
    k1 = D - int(rank1)
    ins = _host_prep(rho_vec)

    from concourse.bass_utils import run_bass_kernel_spmd
    key = (k0, k1)
    if key not in _prog_cache:
        _prog_cache[key] = _build_program(k0, k1)
    nc = _prog_cache[key]
    res = run_bass_kernel_spmd(nc, ins, core_ids=list(range(NCORES)))
    return np.concatenate(
        [np.asarray(res.results[c]["out"]).T.reshape(-1) for c in range(NCORES)]
    ).astype(np.float32)


# revision 7
# speedup vs baseline: 2.0857x; 1.1653x over previous
"""Trainium2 Bass kernel for nn_BESNumEigen3qubitModel (v2).

Same math reduction as v1 (eigenvalues of rho, pt_a(rho), pt_c(rho) per batch
element drive the whole loss), with a faster device algorithm:

  - column-major float layout (f = 64*h + 8*j + i for re/im h of entry (i,j))
    so matrix columns are unit-stride runs -> DVE 2x fp16 mode applies
  - matrices stored fp16, diagonals in a separate f32 tensor
  - 2 full Jacobi sweeps (all 3*32 matrices) + 1 rho-only sweep + a one-shot
    second-order diagonal correction for rho (replaces a 4th sweep)
  - per-round batched rotation-parameter chain (all 4 XOR pairs at once via
    affine access patterns), per-pair column rotations + Hermitian row restore
"""

import numpy as np

D = 8
BATCH = 32768
NCORES = 8
PER_CORE = BATCH // NCORES       # 4096
NTILES = PER_CORE // 128         # 32 rho matrices per partition
NM = 3 * NTILES                  # 96 matrices per partition (type-major)

_f32 = np.float32


# ---------------------------------------------------------------- host prep --

def _gellmann_basis(d):
    mats = []
    for j in range(d):
        for k in range(j + 1, d):
            m = np.zeros((d, d), np.complex128); m[j, k] = 1; m[k, j] = 1
            mats.append(m)
    for j in range(d):
        for k in range(j + 1, d):
            m = np.zeros((d, d), np.complex128); m[j, k] = -1j; m[k, j] = 1j
            mats.append(m)
    for l in range(1, d):
        m = np.zeros((d, d), np.complex128)
        m[np.arange(l), np.arange(l)] = 1
        m[l, l] = -l
        mats.append(np.sqrt(2.0 / (l * (l + 1))) * m)
    return np.stack(mats)


def _pt(m, kind):
    if kind == 'a':
        return np.swapaxes(m.reshape(2, 4, 2, 4), 1, 3).reshape(8, 8)
    return np.swapaxes(m.reshape(4, 2, 4, 2), 1, 3).reshape(8, 8)


def _build_maps():
    """[64, 3*128] f32 map (vec,1) -> col-major floats of rho/pt_a/pt_c,
    and [64, 3*8] diagonal map."""
    G = _gellmann_basis(D)
    basis = list(G) + [np.eye(D) / D]
    M3 = np.zeros((64, 3 * 128), np.float64)
    MD = np.zeros((64, 3 * 8), np.float64)
    for k, A in enumerate(basis):
        for t, At in enumerate((A, _pt(A, 'a'), _pt(A, 'c'))):
            # col-major: f = 64*h + 8*j + i
            M3[k, t * 128:t * 128 + 64] = At.real.T.reshape(-1)
            M3[k, t * 128 + 64:t * 128 + 128] = At.imag.T.reshape(-1)
            MD[k, t * 8:(t + 1) * 8] = np.diagonal(At).real
    return M3.astype(_f32), MD.astype(_f32)


_MAPS = None


def _host_prep(rho_vec):
    global _MAPS
    if _MAPS is None:
        _MAPS = _build_maps()
    M3, MD = _MAPS
    vec = rho_vec.astype(np.float64)
    vec = vec / np.linalg.norm(vec, axis=-1, keepdims=True)
    vec_aug = np.concatenate(
        [vec.astype(_f32), np.ones((vec.shape[0], 1), _f32)], axis=1)
    flat = (vec_aug @ M3).astype(np.float16)               # [B, 384]
    dflat = vec_aug @ MD                                   # [B, 24] f32
    arr = flat.reshape(NCORES, NTILES, 128, 3, 128)
    darr = dflat.reshape(NCORES, NTILES, 128, 3, 8)
    ins = []
    for c in range(NCORES):
        m = np.ascontiguousarray(
            arr[c].transpose(1, 2, 0, 3).reshape(128, NM * 128))
        dg = np.ascontiguousarray(
            darr[c].transpose(1, 2, 0, 3).reshape(128, NM * 8).astype(_f32))
        ins.append({"mats": m, "diag": dg})
    return ins


# ------------------------------------------------------------ device kernel --

def _msb(r):
    return 4 if r >= 4 else (2 if r >= 2 else 1)


# Batcher odd-even mergesort network for 8 elements (19 comparators)
_CE8 = [(0, 1), (2, 3), (4, 5), (6, 7), (0, 2), (1, 3), (4, 6), (5, 7),
        (1, 2), (5, 6), (0, 4), (1, 5), (2, 6), (3, 7), (2, 4), (3, 5),
        (1, 2), (3, 4), (5, 6)]


def _build_program(k0, k1):
    import concourse.bass as bass
    import concourse.bacc as bacc
    import concourse.mybir as mybir
    from concourse.tile import TileContext
    from contextlib import ExitStack

    f32 = mybir.dt.float32
    f16 = mybir.dt.float16
    ALU = mybir.AluOpType
    ACT = mybir.ActivationFunctionType

    nc = bacc.Bacc("TRN2")
    mats_d = nc.dram_tensor("mats", [128, NM * 128], f16, kind="ExternalInput")
    diag_d = nc.dram_tensor("diag", [128, NM * 8], f32, kind="ExternalInput")
    out_d = nc.dram_tensor("out", [128, NTILES], f32, kind="ExternalOutput")

    with ExitStack() as ctx:
        tc = ctx.enter_context(TileContext(nc))
        main = ctx.enter_context(tc.tile_pool(name="main", bufs=1))
        pp = ctx.enter_context(tc.tile_pool(name="pp", bufs=2))
        ep = ctx.enter_context(tc.tile_pool(name="ep", bufs=2))
        cp = ctx.enter_context(tc.tile_pool(name="cp", bufs=2))

        A = main.tile([128, NM, 128], f16, name="A")
        Dg = main.tile([128, NM, 8], f32, name="Dg")
        for ch in range(8):
            nc.sync.dma_start(
                out=A[:, ch * 12:(ch + 1) * 12, :],
                in_=mats_d[:, ch * 12 * 128:(ch + 1) * 12 * 128])
        nc.sync.dma_start(out=Dg[:, :, :], in_=diag_d[:, :])

        eps30 = main.tile([128, 1], f32, name="eps30")
        nc.vector.memset(eps30[:], 1e-30)
        eps35 = main.tile([128, 1], f32, name="eps35")
        nc.vector.memset(eps35[:], 1e-35)

        Aap = A[:]
        pdim = list(Aap.ap[0])
        Dap = Dg[:]
        dpdim = list(Dap.ap[0])

        def aAP(off, dims, M, m0=0):
            return bass.AP(tensor=Aap.tensor,
                           offset=Aap.offset + off + m0 * 128,
                           ap=[list(pdim), [128, M], *[list(d) for d in dims]])

        def dAP(off, dims, M):
            return bass.AP(tensor=Dap.tensor, offset=Dap.offset + off,
                           ap=[list(dpdim), [8, M], *[list(d) for d in dims]])

        def tAP(t, off, dims, M, m0=0):
            # custom inner-dim view of a [128, NM, ...] tile's [m0:m0+M] slice
            tap = t[:]
            return bass.AP(tensor=tap.tensor,
                           offset=tap.offset + off + m0 * tap.ap[1][0],
                           ap=[list(tap.ap[0]), [tap.ap[1][0], M],
                               *[list(d) for d in dims]])

        negone = main.tile([128, 16], f16, name="negone")
        nc.vector.memset(negone[:], -1.0)

        def negbc(Mx):
            nap = negone[:]
            return bass.AP(tensor=nap.tensor, offset=nap.offset,
                           ap=[list(nap.ap[0]), [0, Mx], [0, 2], [1, 8]])

        TT = nc.vector.tensor_tensor
        GT = nc.gpsimd.tensor_tensor
        STT = nc.vector.scalar_tensor_tensor
        GSTT = nc.gpsimd.scalar_tensor_tensor

        # ---------------- one Jacobi round -------------------------------
        def emit_params(r, M, Dv_off, A_off, Mmats, tbdst=None, pre=""):
            """Rotation params for round r over M matrices.

            Returns (csm_t, srp_t, sip_t) tiles [128, Mmats, 4] f32 holding
            per-pair c / sr / si. If tbdst is None, Dg is updated in place
            (dpp/dqq); else tb is accumulated into tbdst AP pair positions.
            """
            hi = _msb(r)
            b1, b2 = [b for b in (1, 2, 4) if b != hi][::-1]  # b1 outer (larger)
            sg_ = lambda b: -b if (r & b) else b
            app = dAP(Dv_off, [[b1, 2], [b2, 2]], M)
            aqq = dAP(Dv_off + r, [[sg_(b1), 2], [sg_(b2), 2]], M)
            sX = [b + 8 * sg_(b) for b in (b1, b2)]
            X = aAP(A_off + 8 * r, [[sX[0], 2], [sX[1], 2]], M)
            Y = aAP(A_off + 8 * r + 64, [[sX[0], 2], [sX[1], 2]], M)

            tiles = {}

            def pt_(tag):
                t = pp.tile([128, Mmats, 4], f32, tag=pre + tag, name=pre + tag)
                tiles[tag] = t
                return (tAP(t, 0, [[2, 2], [1, 2]], M),   # [M, 2, 2] view
                        tAP(t, 0, [[1, 4]], M))           # flat [M, 4] view

            xx, xxf = pt_("xx")
            yy, yyf = pt_("yy")
            m2, m2f = pt_("m2")
            g, gf = pt_("g")
            g2, g2f = pt_("g2")
            s2, s2f = pt_("s2")
            rs, rsf = pt_("rs")
            h, hf = pt_("h")
            ag, agf = pt_("ag")
            den, denf = pt_("den")
            T, Tf = pt_("T")
            sgn, sgnf = pt_("sgn")
            v, vf = pt_("v")
            tb, tbf = pt_("tb")

            TT(xx, X, X, ALU.mult)
            nc.scalar.activation(yy, Y, ACT.Square)
            TT(m2f, xxf, yyf, ALU.add)
            GT(g, app, aqq, ALU.subtract)
            nc.scalar.activation(g2f, gf, ACT.Square)
            STT(s2f, m2f, 4.0, g2f, ALU.mult, ALU.add)
            nc.scalar.activation(rsf, s2f, ACT.Abs_reciprocal_sqrt,
                                 bias=eps30[:])
            nc.scalar.activation(agf, gf, ACT.Abs)
            TT(hf, s2f, rsf, ALU.mult)
            TT(denf, agf, hf, ALU.add)
            nc.vector.reciprocal(Tf, denf)
            nc.scalar.sign(sgnf, gf, bias=eps35[:])
            GT(vf, m2f, Tf, ALU.mult)
            GT(vf, vf, vf, ALU.add)              # v = 2*m2*T
            GT(tbf, vf, sgnf, ALU.mult)          # tb = 2*m2*T*sg (full)

            if tbdst is None:
                GT(app, app, tb, ALU.add)        # Dg[p] += tb
                GT(aqq, aqq, tb, ALU.subtract)   # Dg[q] -= tb
                T2, T2f = pt_("T2")
                t2, t2f = pt_("t2")
                csm, csmf = pt_("csm")
                u2, u2f = pt_("u2")
                urb2, urb2f = pt_("urb2")
                srp, srpf = pt_("srp")
                sip, sipf = pt_("sip")
                nc.scalar.activation(T2f, Tf, ACT.Square)
                STT(t2f, T2f, 4.0, m2f, ALU.mult, ALU.mult)
                nc.scalar.activation(csmf, t2f, ACT.Abs_reciprocal_sqrt,
                                     bias=1.0)
                GT(u2f, Tf, sgnf, ALU.mult)
                STT(urb2f, u2f, 2.0, csmf, ALU.mult, ALU.mult)
                TT(srp, urb2, X, ALU.mult)
                TT(sip, urb2, Y, ALU.mult)
                return tiles["csm"], tiles["srp"], tiles["sip"]
            else:
                tbp = tbdst(0, [[b1, 2], [b2, 2]])
                tbq = tbdst(r, [[sg_(b1), 2], [sg_(b2), 2]])
                GT(tbp, tbp, tb, ALU.add)
                GT(tbq, tbq, tb, ALU.subtract)
                return None

        def emit_round(r, M, Mc):
            """Round r: params over M matrices, col updates over Mc."""
            hi = _msb(r)
            b1, b2 = [b for b in (1, 2, 4) if b != hi][::-1]
            sg_ = lambda b: -b if (r & b) else b
            pairs = [(a, a ^ r) for a in (0, b2, b1, b1 + b2)]

            csm_t, srp_t, sip_t = emit_params(r, M, 0, 0, NM)

            # expansions over the 8-run (packed last dim for DVE 2x)
            c8t = ep.tile([128, NM, 4, 8], f16, tag="c8", name="c8")
            sr8t = ep.tile([128, NM, 4, 8], f16, tag="sr8", name="sr8")
            si8t = ep.tile([128, NM, 4, 8], f16, tag="si8", name="si8")

            def bc8(t, Mx):
                return tAP(t, 0, [[1, 4], [0, 8]], Mx)

            nc.scalar.copy(c8t[:][:, 0:Mc], bc8(csm_t, Mc))
            nc.vector.tensor_copy(sr8t[:][:, 0:Mc], bc8(srp_t, Mc))
            nc.vector.tensor_copy(si8t[:][:, 0:Mc], bc8(sip_t, Mc))

            def scal8(t, k, Mx, m0):
                # [Mx, 2, 8] broadcast of per-pair scalar plane over halves
                return tAP(t, 8 * k, [[0, 2], [1, 8]], Mx, m0)

            # ---------------- per-pair column phase (2 streams) ----------
            half = Mc // 2
            streams = [(0, half, 0), (half, Mc - half, 1)]
            for k, (p, q) in enumerate(pairs):
                d_ = q - p
                PCt = cp.tile([128, NM, 2, 16], f16, tag="PC", name="PC")
                T1t = cp.tile([128, NM, 2, 16], f16, tag="T1", name="T1")
                T2t = cp.tile([128, NM, 2, 16], f16, tag="T2", name="T2")
                for m0, mc, sw in streams:
                    colv = lambda c: aAP(8 * c, [[64, 2], [1, 8]], mc, m0)
                    colJ = lambda c: aAP(8 * c + 64, [[-64, 2], [1, 8]], mc, m0)
                    slot = lambda t, s: tAP(t, 16 * s, [[8, 2], [1, 8]], mc, m0)
                    sel = lambda t, o, st: tAP(t, o, [[st, 2], [1, 8]], mc, m0)
                    TT(slot(T1t, 0), scal8(sr8t, k, mc, m0), colv(q), ALU.mult)
                    TT(slot(T1t, 1), scal8(sr8t, k, mc, m0), colv(p), ALU.mult)
                    TT(slot(T2t, 0), scal8(si8t, k, mc, m0), colJ(q), ALU.mult)
                    TT(slot(T2t, 1), scal8(si8t, k, mc, m0), colJ(p), ALU.mult)
                    TT(slot(PCt, 0), scal8(c8t, k, mc, m0), colv(p), ALU.mult)
                    TT(slot(PCt, 1), scal8(c8t, k, mc, m0), colv(q), ALU.mult)
                    # W+ on sel {(0,re),(1,im)}; W- on sel {(0,im),(1,re)}
                    TT(sel(T1t, 0, 24), sel(T1t, 0, 24), sel(T2t, 0, 24),
                       ALU.add)
                    TT(sel(T1t, 8, 8), sel(T1t, 8, 8), sel(T2t, 8, 8),
                       ALU.subtract)
                    # finals into A columns
                    TT(colv(p), slot(PCt, 0), slot(T1t, 0), ALU.add)
                    TT(colv(q), slot(PCt, 1), slot(T1t, 1), ALU.subtract)
                    # Hermitian row restore (engines swapped per stream)
                    rows_re = aAP(p, [[d_, 2], [8, 8]], mc, m0)
                    cols_re = aAP(8 * p, [[8 * d_, 2], [1, 8]], mc, m0)
                    rows_im = aAP(64 + p, [[d_, 2], [8, 8]], mc, m0)
                    cols_im = aAP(64 + 8 * p, [[8 * d_, 2], [1, 8]], mc, m0)
                    if sw == 0:
                        nc.vector.tensor_copy(rows_re, cols_re)
                        nc.scalar.activation(rows_im, cols_im, ACT.Copy,
                                             scale=-1.0)
                    else:
                        nc.scalar.copy(rows_re, cols_re)
                        TT(rows_im, cols_im, negbc(mc), ALU.mult)

            # ---------------- round-end fixes ----------------------------
            sX = [b + 8 * sg_(b) for b in (b1, b2)]
            ddp = dAP(0, [[b1, 2], [b2, 2]], Mc)
            ddq = dAP(r, [[sg_(b1), 2], [sg_(b2), 2]], Mc)
            nc.gpsimd.tensor_copy(aAP(0, [[9 * b1, 2], [9 * b2, 2]], Mc), ddp)
            nc.gpsimd.tensor_copy(
                aAP(9 * r, [[9 * sg_(b1), 2], [9 * sg_(b2), 2]], Mc), ddq)
            nc.gpsimd.memset(aAP(64, [[9 * b1, 2], [9 * b2, 2]], Mc), 0.0)
            nc.gpsimd.memset(
                aAP(64 + 9 * r, [[9 * sg_(b1), 2], [9 * sg_(b2), 2]], Mc), 0.0)
            # annihilate (p,q) and (q,p), re+im
            sQ = [8 * b + sg_(b) for b in (b1, b2)]
            nc.gpsimd.memset(aAP(8 * r, [[sX[0], 2], [sX[1], 2]], Mc), 0.0)
            nc.gpsimd.memset(aAP(8 * r + 64, [[sX[0], 2], [sX[1], 2]], Mc), 0.0)
            nc.gpsimd.memset(aAP(r, [[sQ[0], 2], [sQ[1], 2]], Mc), 0.0)
            nc.gpsimd.memset(aAP(r + 64, [[sQ[0], 2], [sQ[1], 2]], Mc), 0.0)

        # ---------------- sweeps ----------------------------------------
        for r in range(1, 8):
            emit_round(r, NM, NM)
        for r in range(1, 6):
            emit_round(r, NM, NM)
        for r in range(1, 8):
            emit_round(r, NTILES, NTILES)

        # ---------------- one-shot correction (all matrices) -------------
        TB = main.tile([128, NM, 8], f32, name="TB")
        nc.vector.memset(TB[:], 0.0)
        Tap = TB[:]
        tpdim = list(Tap.ap[0])

        for r in range(1, 8):
            def tbdst(off, dims):
                return bass.AP(tensor=Tap.tensor, offset=Tap.offset + off,
                               ap=[list(tpdim), [8, NM],
                                   *[list(d) for d in dims]])
            emit_params(r, NM, 0, 0, NM, tbdst=tbdst)

        TT(Dg[:, :, :], Dg[:, :, :], TB[:], ALU.add)

        # ---------------- rho diagonal sort ------------------------------
        tmin = main.tile([128, NTILES], f32, name="tmin")[:]
        dg8 = Dg[:]
        for (i, j) in _CE8:
            di = dg8[:, 0:NTILES, i]
            dj = dg8[:, 0:NTILES, j]
            TT(tmin, di, dj, ALU.min)
            TT(dj, di, dj, ALU.max)
            nc.gpsimd.tensor_copy(di, tmin)

        # ---------------- pt_a / pt_c min & max --------------------------
        mn = main.tile([128, 2 * NTILES], f32, name="mn")[:]
        mx = main.tile([128, 2 * NTILES], f32, name="mx")[:]
        ptd = dg8[:, NTILES:NM, :]
        nc.vector.tensor_reduce(mn, ptd, mybir.AxisListType.X, ALU.min)
        nc.vector.tensor_reduce(mx, ptd, mybir.AxisListType.X, ALU.max)
        mu_min = mn[:, 0:NTILES]
        mu_max = mx[:, 0:NTILES]
        nu_min = mn[:, NTILES:2 * NTILES]
        nu_max = mx[:, NTILES:2 * NTILES]

        # ---------------- loss assembly ----------------------------------
        def L(name):
            return main.tile([128, NTILES], f32, tag=name, name=name)[:]

        w_min = dg8[:, 0:NTILES, 0]
        w_max = dg8[:, 0:NTILES, 7]
        b0, b1_, acc, t1, t2_, t3 = (L("b0"), L("b1"), L("acc"), L("t1"),
                                     L("t2x"), L("t3"))

        nc.vector.tensor_scalar(b0, w_min, -8.0, 1.0, ALU.mult, ALU.add)
        nc.vector.reciprocal(b0, b0)
        nc.vector.tensor_scalar(b1_, w_max, -8.0, 1.0, ALU.mult, ALU.add)
        nc.vector.reciprocal(b1_, b1_)

        assert 1 <= k0 <= 8 and 1 <= k1 <= 8
        nc.gpsimd.tensor_copy(t1, dg8[:, 0:NTILES, 0])
        for i in range(1, k0):
            TT(t1, t1, dg8[:, 0:NTILES, i], ALU.add)
        nc.gpsimd.tensor_copy(t2_, dg8[:, 0:NTILES, 7])
        for i in range(6, 7 - k1, -1):
            TT(t2_, t2_, dg8[:, 0:NTILES, i], ALU.add)
        nc.vector.tensor_scalar(t1, t1, -k0 / 8.0, None, ALU.add)
        TT(t1, t1, b0, ALU.mult)
        nc.vector.tensor_scalar(t2_, t2_, -k1 / 8.0, None, ALU.add)
        TT(t2_, t2_, b1_, ALU.mult)
        TT(t1, t1, t2_, ALU.add)
        nc.vector.tensor_scalar(t1, t1, (k0 + k1) / 8.0, None, ALU.add)
        TT(acc, t1, t1, ALU.mult)
        for beta, ext in ((b0, mu_min), (b1_, mu_max), (b0, nu_min),
                          (b1_, nu_max)):
            nc.vector.tensor_scalar(t3, ext, -0.125, None, ALU.add)
            TT(t3, t3, beta, ALU.mult)
            nc.vector.tensor_scalar(t3, t3, 0.125, None, ALU.add)
            TT(t3, t3, t3, ALU.mult)
            TT(acc, acc, t3, ALU.add)

        nc.sync.dma_start(out=out_d[:, :], in_=acc)

    nc.finalize()
    return nc


_prog_cache = {}


def kernel(rho_vec, rank0, rank1):
    rho_vec = np.asarray(rho_vec, dtype=np.float32)
    k0 = D - int(rank0)
    k1 = D - int(rank1)
    ins = _host_prep(rho_vec)

    from concourse.bass_utils import run_bass_kernel_spmd
    key = (k0, k1)
    if key not in _prog_cache:
        _prog_cache[key] = _build_program(k0, k1)
    nc = _prog_cache[key]
    res = run_bass_kernel_spmd(nc, ins, core_ids=list(range(NCORES)))
    return np.concatenate(
        [np.asarray(res.results[c]["out"]).T.reshape(-1) for c in range(NCORES)]
    ).astype(np.float32)
